# revision 6
# baseline (speedup 1.0000x reference)
# Trainium2 Bass kernel for nn_LogitsNew (dense_mlp).
#
#   u = gelu(x @ W_proj + b_proj)                       [B, D]
#   logits = (u @ W_u)[:, None, :] + ee @ W_e           [B, N, C]
#
# Sharding: data-parallel over batch B across 8 cores (4 batches/core).
#
# All data moves as bf16 (host-cast; ~0.4% norm rel err, gate is 2e-2):
# halves HBM traffic vs fp32 to ~10MB/core. ee is transposed on the host
# into k-slice-major lhsT layout, eliminating all on-device PE transposes
# of ee. Output is stored bf16 and upcast on the host.
#
# Per core:
#   - warmup: a few dummy matmuls so the PE HAM clock-gate opens early.
#   - phase 1: k-outer accumulation over m-tiles 0..3 (8 PSUM banks).
#     Consumption is 2 matmuls per 256KB k-slice pair (eeT + W_e), so the
#     PE runs dense as soon as the first k-slices land (~1.3us) instead
#     of waiting for all of W_e.
#   - utterance path: z = x@W_proj + b (K=1 ones matmul for bias),
#     u = Gelu(z) on ACT, uT via PE transpose, y = uT.T@W_u,
#     y broadcast across partitions on gpsimd.
#   - phase 3: k-inner per m-tile for m-tiles 4..7, drains fused with the
#     y-add on DVE; phase-1 tiles get their y-add in an overlapped
#     epilogue. Stores stream out per-tile as bf16.
#
# DMA rings: SP (sync) carries W_e/W_proj k-slices + stores; ACT (scalar)
# carries xt/b, eeT k-slices, W_u, and the y SBUF->SBUF roundtrip.

import sys

if "/opt/trn_rl_repo" not in sys.path:
    sys.path.insert(0, "/opt/trn_rl_repo")

import numpy as np
import ml_dtypes

import concourse.bass as bass
import concourse.mybir as mybir
import concourse.tile as tile
from concourse import bacc
from concourse.bass_utils import run_bass_kernel_spmd
from concourse.masks import make_identity

P = 128
B, N, D, C = 32, 256, 1024, 1024
NCORES = 8
BPC = B // NCORES          # batches per core
KT = D // P                # 8 k-tiles over the contraction dim
FD = 512                   # matmul moving free dim (one PSUM bank of fp32)
NT = N // P                # 2 n-tiles per batch
MT = BPC * NT              # 8 m-tiles per core

F32 = mybir.dt.float32
BF16 = mybir.dt.bfloat16
GELU = mybir.ActivationFunctionType.Gelu
BF = ml_dtypes.bfloat16

_CACHE = {}


def _build():
    if "nc" in _CACHE:
        return _CACHE["nc"]

    nc = bacc.Bacc("TRN2", target_bir_lowering=False, debug=False, num_devices=NCORES)

    # host-packed inputs (see kernel() for the packing)
    eet = nc.dram_tensor("eet", [KT, P, MT * P], BF16, kind="ExternalInput").ap()
    we = nc.dram_tensor("we", [KT, P, C], BF16, kind="ExternalInput").ap()
    wp = nc.dram_tensor("wp", [KT, P, C], BF16, kind="ExternalInput").ap()
    wu = nc.dram_tensor("wu", [KT, P, C], BF16, kind="ExternalInput").ap()
    xt = nc.dram_tensor("xt", [P, KT, BPC], BF16, kind="ExternalInput").ap()
    bp = nc.dram_tensor("bp", [1, D], BF16, kind="ExternalInput").ap()
    out = nc.dram_tensor("logits", [MT, P, C], BF16, kind="ExternalOutput").ap()

    with tile.TileContext(nc) as tc:
        with (
            tc.tile_pool(name="const", bufs=1) as cpool,
            tc.tile_pool(name="weights", bufs=1) as wpool,
            tc.tile_pool(name="o32", bufs=1) as o32pool,
            tc.tile_pool(name="obf", bufs=1) as obfpool,
            tc.tile_pool(name="mm_ps", bufs=8, space="PSUM") as mm_ps,
        ):
            # ---- constants ----
            ident_f = cpool.tile([P, P], F32)
            make_identity(nc, ident_f)
            ident = cpool.tile([P, P], BF16)
            nc.scalar.copy(ident, ident_f)
            ones_f = cpool.tile([1, BPC], F32)
            nc.gpsimd.memset(ones_f, 1.0)
            ones = cpool.tile([1, BPC], BF16)
            nc.scalar.copy(ones, ones_f)

            # ---- load DMAs: both rings fill in consumption order ----
            xt_sb = cpool.tile([P, KT, BPC], BF16)
            nc.scalar.dma_start(xt_sb, xt)
            b_sb = cpool.tile([1, D], BF16)
            nc.scalar.dma_start(b_sb, bp)

            we_sb = wpool.tile([P, KT, C], BF16)
            eet_sb = wpool.tile([P, KT, MT * P], BF16)
            wp_sb = wpool.tile([P, KT, C], BF16)
            wu_sb = wpool.tile([P, KT, C], BF16)
            for ko in range(KT):
                nc.sync.dma_start(we_sb[:, ko, :], we[ko])
                nc.scalar.dma_start(eet_sb[:, ko, :], eet[ko])
            for ko in range(KT):
                nc.sync.dma_start(wp_sb[:, ko, :], wp[ko])
                nc.scalar.dma_start(wu_sb[:, ko, :], wu[ko])

            # ---- PE warmup: keep HAM busy while first k-slices land ----
            dummy = mm_ps.tile([P, FD], F32, tag="mm", name="dummy")
            for _ in range(12):
                nc.tensor.matmul(dummy[:, :P], ident, ident, start=True, stop=True)

            # ---- phase 1: k-outer accumulation over m-tiles 0..3 ----
            ph1 = {}
            for m in range(4):
                for h in range(2):
                    ph1[m, h] = mm_ps.tile([P, FD], F32, tag="mm", name=f"p1_{m}_{h}")
            for ko in range(KT):
                for m in range(4):
                    ms = slice(m * P, (m + 1) * P)
                    for h in range(2):
                        nc.tensor.matmul(
                            ph1[m, h],
                            eet_sb[:, ko, ms],
                            we_sb[:, ko, h * FD : (h + 1) * FD],
                            start=(ko == 0),
                            stop=(ko == KT - 1),
                        )

            # drains (no y yet): PSUM -> f32 SBUF on DVE
            o32 = {}
            for m in range(4):
                o32[m] = o32pool.tile([P, C], F32, tag=f"f{m}", name=f"o32_{m}")
                for h in range(2):
                    nc.vector.tensor_copy(
                        o32[m][:, h * FD : (h + 1) * FD], ph1[m, h]
                    )

            # ---- utterance path ----
            # z = x @ W_proj + b, via K=1 ones matmul for the bias
            u32 = cpool.tile([BPC, C], F32)
            zps = {}
            for h in range(2):
                cs = slice(h * FD, (h + 1) * FD)
                zp = mm_ps.tile([P, FD], F32, tag="mm", name=f"z_{h}")
                zps[h] = zp
                for ko in range(KT):
                    nc.tensor.matmul(
                        zp[:BPC], xt_sb[:, ko, :], wp_sb[:, ko, cs],
                        start=(ko == 0), stop=False,
                    )
                nc.tensor.matmul(
                    zp[:BPC], ones[:1, :BPC], b_sb[:1, cs],
                    start=False, stop=True,
                )
            for h in range(2):
                cs = slice(h * FD, (h + 1) * FD)
                nc.scalar.activation(u32[:, cs], zps[h][:BPC], GELU)

            # uT via PE transpose
            uT = cpool.tile([P, KT, BPC], BF16)
            for ko in range(KT):
                tp = mm_ps.tile([P, FD], F32, tag="mm", name=f"tp_{ko}")
                nc.tensor.transpose(
                    tp[:, :BPC],
                    u32[:BPC, ko * P : (ko + 1) * P],
                    ident_f[:BPC, :BPC],
                )
                nc.scalar.copy(uT[:, ko, :], tp[:, :BPC])

            # y = u @ W_u
            y_sb = cpool.tile([BPC, C], F32)
            for h in range(2):
                cs = slice(h * FD, (h + 1) * FD)
                yp = mm_ps.tile([P, FD], F32, tag="mm", name=f"y_{h}")
                for ko in range(KT):
                    nc.tensor.matmul(
                        yp[:BPC], uT[:, ko, :], wu_sb[:, ko, cs],
                        start=(ko == 0), stop=(ko == KT - 1),
                    )
                nc.vector.tensor_copy(y_sb[:, cs], yp[:BPC])

            # broadcast y across partitions: SBUF roundtrip to partition 0,
            # then gpsimd partition_broadcast, in phase-3 consumption order
            y_row = cpool.tile([1, BPC, C], F32)
            nc.scalar.dma_start(y_row, y_sb)
            ybc = cpool.tile([P, BPC, C], F32)
            for b2 in (2, 3, 0, 1):
                nc.gpsimd.partition_broadcast(ybc[:, b2, :], y_row[:1, b2, :])

            # ---- phase 3: k-inner per m-tile 4..7, fused drain + y add ----
            obf = {}

            def fused_tile(mt):
                b = mt // NT
                mps = [
                    mm_ps.tile([P, FD], F32, tag="mm", name=f"p3_{mt}_{h}")
                    for h in range(2)
                ]
                ms = slice(mt * P, (mt + 1) * P)
                for ko in range(KT):
                    for h in range(2):
                        nc.tensor.matmul(
                            mps[h],
                            eet_sb[:, ko, ms],
                            we_sb[:, ko, h * FD : (h + 1) * FD],
                            start=(ko == 0),
                            stop=(ko == KT - 1),
                        )
                o = obfpool.tile([P, C], BF16, tag=f"o{mt}", name=f"obf_{mt}")
                obf[mt] = o
                for h in range(2):
                    cs = slice(h * FD, (h + 1) * FD)
                    nc.vector.tensor_add(o[:, cs], mps[h], ybc[:, b, cs])
                nc.sync.dma_start(out[mt], o)

            def epilogue_tile(mt):
                b = mt // NT
                o = obfpool.tile([P, C], BF16, tag=f"o{mt}", name=f"obf_{mt}")
                obf[mt] = o
                nc.vector.tensor_add(o, o32[mt], ybc[:, b, :])
                nc.sync.dma_start(out[mt], o)

            fused_tile(4)
            fused_tile(5)
            epilogue_tile(0)
            epilogue_tile(1)
            fused_tile(6)
            epilogue_tile(2)
            epilogue_tile(3)
            fused_tile(7)

    nc.compile()
    _CACHE["nc"] = nc
    return nc


def run(inputs, trace=False, **kwargs):
    nc = _build()
    x = np.asarray(inputs["encoded_utterance"], np.float32)
    ee = np.asarray(inputs["element_embeddings"], np.float32)
    w = np.asarray(inputs["weight_matrix"], np.float32)
    wp = np.asarray(inputs["W_proj"], np.float32)
    bp = np.asarray(inputs["b_proj"], np.float32).reshape(1, D)

    # shared weight packs (k-slice major, bf16)
    wu_p = np.ascontiguousarray(w[:D].reshape(KT, P, C)).astype(BF)
    we_p = np.ascontiguousarray(w[D:].reshape(KT, P, C)).astype(BF)
    wp_p = np.ascontiguousarray(wp.reshape(KT, P, C)).astype(BF)
    bp_p = bp.astype(BF)

    in_maps = []
    for i in range(NCORES):
        bs = slice(i * BPC, (i + 1) * BPC)
        # eeT: [4, 256, D] -> [m=1024, D] -> [D, m] -> [KT, P, m]
        ee_c = ee[bs].reshape(BPC * N, D)
        eet_p = np.ascontiguousarray(ee_c.T.reshape(KT, P, MT * P)).astype(BF)
        # xT: [4, D] -> [D, 4] -> [KT, P, 4] -> [P, KT, 4]
        xt_p = np.ascontiguousarray(
            x[bs].T.reshape(KT, P, BPC).transpose(1, 0, 2)
        ).astype(BF)
        in_maps.append(
            {
                "eet": eet_p,
                "we": we_p,
                "wp": wp_p,
                "wu": wu_p,
                "xt": xt_p,
                "bp": bp_p,
            }
        )

    res = run_bass_kernel_spmd(
        nc, in_maps, core_ids=list(range(NCORES)), trace=trace, **kwargs
    )
    full = np.concatenate(
        [
            np.asarray(r["logits"]).astype(np.float32).reshape(BPC, N, C)
            for r in res.results
        ],
        axis=0,
    )
    return full, res


def kernel(**inputs) -> np.ndarray:
    return run(inputs, trace=False)[0]
